# revision 12
# baseline (speedup 1.0000x reference)
"""DecoderVAE single-step fused kernel for one TRN2 chip (8 NeuronCores).

Sharding: data-parallel over batch (32/core) for attention + decoder front;
tensor-parallel GRU (144 GRU-state rows/core) and output projection
(4000 vocab rows/core).  Two on-device AllGathers stitch the phases
(x: 512x256, h_new: 1152x256).

All matmuls run in float32r (4-byte storage, full-rate PE at N>=256,
~2e-4 relative error vs f32).
"""
import sys
import numpy as np

for _p in ("/opt/trn_rl_repo", "/root/.axon_site/_ro/trn_rl_repo"):
    if _p not in sys.path:
        sys.path.append(_p)

import concourse.bacc as bacc
import concourse.mybir as mybir
from concourse import tile
from concourse.bass_utils import run_bass_kernel_spmd
from concourse.masks import make_identity

dt = mybir.dt
AF = mybir.ActivationFunctionType
ALU = mybir.AluOpType

# problem dims (hardcoded)
B = 256          # batch
H = 512          # hidden
Z = 128
E = 512
D = 1152         # GRU state
V = 32000
N = 128          # encoder nodes
G3 = 3456

NC = 8           # cores
BSH = B // NC    # 32 batch rows per core
NB = 4           # batch rows per attention tile
NT = BSH // NB   # 8 attention tiles per encoder
HC = H // 128    # 4
EHC = (E + H) // 128  # 8
DC = D // 128    # 9
DSH = D // NC    # 144 GRU rows per core
VSH = V // NC    # 4000 vocab rows per core
VC = 32          # vocab chunks per core (31x128 + 32)
SUBS = ((0, 128), (128, 16))  # DSH split into partition tiles

F32 = dt.float32
F32R = dt.float32r


def build():
    nc = bacc.Bacc(None, target_bir_lowering=False, num_devices=NC)

    def param(name, shape):
        return nc.declare_dram_parameter(name, list(shape), F32, isOutput=False)

    enc_e = param("enc_e", (BSH, N, H))
    enc_s = param("enc_s", (BSH, N, H))
    mask_e = param("mask_e", (BSH, N))
    mask_s = param("mask_s", (BSH, N))
    w1et = param("w1et", (H, H))
    w1st = param("w1st", (H, H))
    w2et = param("w2et", (D, H))
    w2st = param("w2st", (D, H))
    vte4 = param("vte4", (H, NB))
    vts4 = param("vts4", (H, NB))
    wct = param("wct", (E + H, H))
    planwt = param("planwt", (D, 2))
    planb = param("planb", (2, 1))
    bc = param("bc", (H,))
    ph_T = param("ph_T", (D, B))
    ph_myb = param("ph_myb", (D, BSH))
    ph_myd = param("ph_myd", (DSH, B))
    pyt = param("pyt", (E, BSH))
    wih = param("wih", (3, H, DSH))
    whh = param("whh", (3, D, DSH))
    bih = param("bih", (3, DSH))
    bhh = param("bhh", (3, DSH))
    woutt = param("woutt", (D, VSH))
    bout = param("bout", (VSH,))
    diag4 = param("diag4", (NB, NB * N))

    logitsT = nc.declare_dram_parameter("logitsT", [VSH, B], F32, isOutput=True)
    h_new_sh = nc.declare_dram_parameter("h_new_sh", [DSH, B], F32, isOutput=True)
    plan_sh = nc.declare_dram_parameter("plan_sh", [BSH, 2], F32, isOutput=True)

    with tile.TileContext(nc) as tc:
        with (
            tc.tile_pool(name="wres", bufs=1) as wres,
            tc.tile_pool(name="w2p", bufs=1) as w2p,
            tc.tile_pool(name="enc", bufs=2) as encp,
            tc.tile_pool(name="encT", bufs=2) as encTp,
            tc.tile_pool(name="tanh", bufs=2) as tanhp,
            tc.tile_pool(name="atts", bufs=2) as atts,
            tc.tile_pool(name="mid", bufs=1) as mid,
            tc.tile_pool(name="woutp", bufs=2) as woutp,
            tc.tile_pool(name="loutp", bufs=2) as loutp,
            tc.tile_pool(name="tp", bufs=3, space="PSUM") as tp,
            tc.tile_pool(name="mm", bufs=2, space="PSUM") as mm,
            tc.tile_pool(name="up", bufs=1, space="PSUM") as up,
            tc.tile_pool(name="ctxp", bufs=2, space="PSUM") as ctxp,
            tc.tile_pool(name="dram", bufs=1, space="DRAM") as dram,
        ):
            # ---- collective bounce buffers ----
            agx_in = dram.tile([H * BSH], F32R)
            agx_out = dram.tile([H * B], F32R, addr_space="Shared")
            agh_in = dram.tile([DSH * B], F32R)
            agh_out = dram.tile([D * B], F32R, addr_space="Shared")

            # ---- resident weights / constants ----
            ident_f = wres.tile([128, 128], F32)
            make_identity(nc, ident_f[:])
            ident_r = wres.tile([128, 128], F32R)
            nc.vector.tensor_copy(ident_r[:], ident_f[:])
            zeros16 = wres.tile([128, NB * NB], F32)
            nc.vector.memset(zeros16[:], 0.0)

            w1et_sb = wres.tile([128, HC, H], F32R)
            nc.gpsimd.dma_start(out=w1et_sb[:], in_=w1et[:].rearrange("(k p) h -> p k h", p=128))
            w1st_sb = wres.tile([128, HC, H], F32R)
            nc.gpsimd.dma_start(out=w1st_sb[:], in_=w1st[:].rearrange("(k p) h -> p k h", p=128))
            vte4_sb = wres.tile([128, HC, NB], F32R)
            nc.gpsimd.dma_start(out=vte4_sb[:], in_=vte4[:].rearrange("(k p) j -> p k j", p=128))
            vts4_sb = wres.tile([128, HC, NB], F32R)
            nc.gpsimd.dma_start(out=vts4_sb[:], in_=vts4[:].rearrange("(k p) j -> p k j", p=128))
            wct_sb = wres.tile([128, EHC, H], F32R)
            nc.gpsimd.dma_start(out=wct_sb[:], in_=wct[:].rearrange("(k p) h -> p k h", p=128))
            planwt_sb = wres.tile([128, DC, 2], F32R)
            nc.gpsimd.dma_start(out=planwt_sb[:], in_=planwt[:].rearrange("(k p) j -> p k j", p=128))
            phT_sb = wres.tile([128, DC, B], F32R)
            nc.gpsimd.dma_start(out=phT_sb[:], in_=ph_T[:].rearrange("(k p) b -> p k b", p=128))
            phmyb_sb = wres.tile([128, DC, BSH], F32R)
            nc.gpsimd.dma_start(out=phmyb_sb[:], in_=ph_myb[:].rearrange("(k p) b -> p k b", p=128))
            wih_sb = wres.tile([128, 3, HC, DSH], F32R)
            nc.gpsimd.dma_start(out=wih_sb[:], in_=wih[:].rearrange("g (k p) m -> p g k m", p=128))
            whh_sb = wres.tile([128, 3, DC, DSH], F32R)
            nc.gpsimd.dma_start(out=whh_sb[:], in_=whh[:].rearrange("g (k p) m -> p g k m", p=128))

            maskE_sb = wres.tile([NB, NT, N], F32)
            nc.sync.dma_start(out=maskE_sb[:], in_=mask_e[:].rearrange("(t j) n -> j t n", j=NB))
            maskS_sb = wres.tile([NB, NT, N], F32)
            nc.sync.dma_start(out=maskS_sb[:], in_=mask_s[:].rearrange("(t j) n -> j t n", j=NB))
            bc_sb = wres.tile([128, HC], F32)
            nc.sync.dma_start(out=bc_sb[:], in_=bc[:].rearrange("(k p) -> p k", p=128))
            planb_sb = wres.tile([2, 1], F32)
            nc.sync.dma_start(out=planb_sb[:], in_=planb[:])
            phmyd_hi = wres.tile([128, B], F32)
            nc.sync.dma_start(out=phmyd_hi[:], in_=ph_myd[0:128, :])
            phmyd_lo = wres.tile([16, B], F32)
            nc.sync.dma_start(out=phmyd_lo[:], in_=ph_myd[128:144, :])

            # gate biases: [sub, 3] layout
            bih_hi = wres.tile([128, 3], F32)
            nc.sync.dma_start(out=bih_hi[:], in_=bih[:, 0:128].rearrange("g m -> m g"))
            bih_lo = wres.tile([16, 3], F32)
            nc.sync.dma_start(out=bih_lo[:], in_=bih[:, 128:144].rearrange("g m -> m g"))
            bhh_hi = wres.tile([128, 3], F32)
            nc.sync.dma_start(out=bhh_hi[:], in_=bhh[:, 0:128].rearrange("g m -> m g"))
            bhh_lo = wres.tile([16, 3], F32)
            nc.sync.dma_start(out=bhh_lo[:], in_=bhh[:, 128:144].rearrange("g m -> m g"))
            bsum_hi = wres.tile([128, 3], F32)
            nc.vector.tensor_add(bsum_hi[:], bih_hi[:], bhh_hi[:])
            bsum_lo = wres.tile([16, 3], F32)
            nc.vector.tensor_add(bsum_lo[:], bih_lo[:], bhh_lo[:])

            diag4_sb = wres.tile([NB, NB * N], F32)
            nc.sync.dma_start(out=diag4_sb[:], in_=diag4[:])
            bout_sb = wres.tile([128, VC], F32)
            nc.sync.dma_start(
                out=bout_sb[:, 0:31], in_=bout[0:3968].rearrange("(m p) -> p m", p=128)
            )
            nc.sync.dma_start(
                out=bout_sb[0:32, 31:32], in_=bout[3968:4000].rearrange("(m p) -> p m", p=32)
            )

            # ---- phase W: dt (attention query proj) + plan ----
            w2et_sb = w2p.tile([128, DC, H], F32R, tag="w2")
            nc.gpsimd.dma_start(out=w2et_sb[:], in_=w2et[:].rearrange("(k p) h -> p k h", p=128))
            dtTe = wres.tile([128, HC, BSH], F32)
            for hc in range(HC):
                ps = mm.tile([128, 512], F32, tag="mm")
                for k in range(DC):
                    nc.tensor.matmul(
                        ps[:, 0:BSH],
                        w2et_sb[:, k, hc * 128:(hc + 1) * 128],
                        phmyb_sb[:, k, :],
                        start=(k == 0), stop=(k == DC - 1),
                    )
                nc.scalar.copy(dtTe[:, hc, :], ps[:, 0:BSH])
            w2st_sb = w2p.tile([128, DC, H], F32R, tag="w2")
            nc.gpsimd.dma_start(out=w2st_sb[:], in_=w2st[:].rearrange("(k p) h -> p k h", p=128))
            dtTs = wres.tile([128, HC, BSH], F32)
            for hc in range(HC):
                ps = mm.tile([128, 512], F32, tag="mm")
                for k in range(DC):
                    nc.tensor.matmul(
                        ps[:, 0:BSH],
                        w2st_sb[:, k, hc * 128:(hc + 1) * 128],
                        phmyb_sb[:, k, :],
                        start=(k == 0), stop=(k == DC - 1),
                    )
                nc.scalar.copy(dtTs[:, hc, :], ps[:, 0:BSH])

            # plan logits (softmax applied per-tile below)
            ps = mm.tile([128, 512], F32, tag="mm")
            for k in range(DC):
                nc.tensor.matmul(
                    ps[0:2, 0:BSH], planwt_sb[:, k, :], phmyb_sb[:, k, :],
                    start=(k == 0), stop=(k == DC - 1),
                )
            pl_sb = mid.tile([2, BSH], F32)
            nc.scalar.activation(pl_sb[:], ps[0:2, 0:BSH], AF.Identity, bias=planb_sb[:])

            # y_ctxT = [prev_yT ; contextT] (context columns filled per-tile)
            yctxT = mid.tile([128, EHC, BSH], F32R)
            nc.gpsimd.dma_start(
                out=yctxT[:, 0:4, :], in_=pyt[:].rearrange("(k p) b -> p k b", p=128)
            )

            # ---- attention (both encoders), batch tiles of 4 ----
            srcs = {
                "e": (enc_e, w1et_sb, vte4_sb, dtTe, maskE_sb),
                "s": (enc_s, w1st_sb, vts4_sb, dtTs, maskS_sb),
            }
            for t in range(NT):
                ctx4 = {}
                for key in ("e", "s"):
                    enc_p, w1_sb, vt_sb, dtT, mask_sb = srcs[key]
                    enc_sb = encp.tile([128, NB, H], F32R, tag="enc", name=f"enc_{key}{t}")
                    nc.gpsimd.dma_start(
                        out=enc_sb[:],
                        in_=enc_p[t * NB:(t + 1) * NB].rearrange("b n h -> n b h"),
                    )
                    # transpose to encT [h' part][kc, b, n]
                    encT_sb = encTp.tile([128, HC, NB, N], F32R, tag="encT",
                                         name=f"encT_{key}{t}")
                    for kc in range(HC):
                        tps = tp.tile([128, 512], F32R, tag="tp", name=f"tpt_{key}{t}{kc}")
                        for j in range(NB):
                            nc.tensor.transpose(
                                tps[:, j * N:(j + 1) * N],
                                enc_sb[:, j, kc * 128:(kc + 1) * 128],
                                ident_r[:],
                            )
                        nc.scalar.copy(encT_sb[:, kc, :, :], tps[:])
                    # et = W1T.T @ encT ; T = tanh(et + dt)
                    T_sb = tanhp.tile([128, HC, NB, N], F32R, tag="tanh",
                                      name=f"T_{key}{t}")
                    for hc in range(HC):
                        ps = mm.tile([128, 512], F32, tag="mm", name=f"et_{key}{t}{hc}")
                        for kc in range(HC):
                            nc.tensor.matmul(
                                ps[:],
                                w1_sb[:, kc, hc * 128:(hc + 1) * 128],
                                encT_sb[:, kc, :, :],
                                start=(kc == 0), stop=(kc == HC - 1),
                            )
                        for j in range(NB):
                            bidx = t * NB + j
                            nc.scalar.activation(
                                T_sb[:, hc, j, :], ps[:, j * N:(j + 1) * N],
                                AF.Tanh, bias=dtT[:, hc, bidx:bidx + 1],
                            )
                    # u = vt . T   (4 identical output rows, diagonal extract)
                    ups = up.tile([NB, 512], F32, tag="u", name=f"u_{key}{t}")
                    for hc in range(HC):
                        nc.tensor.matmul(
                            ups[:], vt_sb[:, hc, :], T_sb[:, hc, :, :],
                            start=(hc == 0), stop=(hc == HC - 1),
                        )
                    usel = atts.tile([NB, NB * N], F32, tag="usel", name=f"usel_{key}{t}")
                    nc.vector.tensor_mul(usel[:], ups[:], diag4_sb[:])
                    u4 = atts.tile([NB, N], F32, tag="u4", name=f"u4_{key}{t}")
                    nc.vector.reduce_sum(
                        u4[:], usel[:].rearrange("p (b n) -> p n b", b=NB),
                        axis=mybir.AxisListType.X,
                    )
                    # softmax with additive mask
                    nc.vector.tensor_add(u4[:], u4[:], mask_sb[:, t, :])
                    negmax = atts.tile([NB, 1], F32, tag="nm", name=f"nm_{key}{t}")
                    nc.vector.reduce_max(negmax[:], u4[:], axis=mybir.AxisListType.X, negate=True)
                    aw = atts.tile([NB, N], F32, tag="aw", name=f"aw_{key}{t}")
                    sums = atts.tile([NB, 1], F32, tag="sm", name=f"sm_{key}{t}")
                    nc.scalar.activation(aw[:], u4[:], AF.Exp, bias=negmax[:], accum_out=sums[:])
                    rinv = atts.tile([NB, 1], F32, tag="ri", name=f"ri_{key}{t}")
                    nc.vector.reciprocal(rinv[:], sums[:])
                    nc.vector.tensor_scalar_mul(aw[:], aw[:], rinv[:])
                    # awT, block-diagonal [n, b_local*NB + j] (zeros off-diag)
                    tps2 = tp.tile([128, 512], F32, tag="tp", name=f"tpa_{key}{t}")
                    nc.tensor.transpose(tps2[:, 0:NB], aw[:], ident_f[0:NB, 0:NB])
                    awT4z = atts.tile([128, NB, NB], F32R, tag="awT", name=f"awT_{key}{t}")
                    nc.vector.tensor_copy(awT4z[:].rearrange("p a b -> p (a b)"), zeros16[:])
                    for j in range(NB):
                        nc.scalar.copy(awT4z[:, j, j:j + 1], tps2[:, j:j + 1])
                    # ctx4[j, :] = sum_n aw[j, n] * enc[j, n, :]
                    cps = ctxp.tile([NB, 512], F32, tag="ctx", name=f"ctx_{key}{t}")
                    for j in range(NB):
                        nc.tensor.matmul(
                            cps[:], awT4z[:, j, :], enc_sb[:, j, :],
                            start=(j == 0), stop=(j == NB - 1),
                        )
                    c4 = mid.tile([NB, 512], F32, tag=f"c4{key}", name=f"c4_{key}{t}")
                    nc.scalar.copy(c4[:], cps[:])
                    ctx4[key] = c4

                # per-tile plan softmax (2-way -> sigmoid of diff)
                tpp = tp.tile([128, 512], F32, tag="tp", name=f"tpp{t}")
                nc.tensor.transpose(
                    tpp[0:NB, 0:2], pl_sb[:, t * NB:(t + 1) * NB], ident_f[0:2, 0:2]
                )
                planT4 = atts.tile([NB, 2], F32, tag="pT", name=f"pT{t}")
                nc.scalar.copy(planT4[:], tpp[0:NB, 0:2])
                d01 = atts.tile([NB, 1], F32, tag="d01", name=f"d01_{t}")
                nc.vector.tensor_sub(d01[:], planT4[:, 0:1], planT4[:, 1:2])
                plan0 = atts.tile([NB, 1], F32, tag="p0", name=f"p0_{t}")
                nc.scalar.activation(plan0[:], d01[:], AF.Sigmoid)
                plan_out4 = atts.tile([NB, 2], F32, tag="po", name=f"po_{t}")
                nc.vector.tensor_copy(plan_out4[:, 0:1], plan0[:])
                nc.scalar.activation(plan_out4[:, 1:2], plan0[:], AF.Identity,
                                     bias=1.0, scale=-1.0)
                nc.sync.dma_start(out=plan_sh[t * NB:(t + 1) * NB, :], in_=plan_out4[:])

                # context4 = ctx_s + plan0 * (ctx_e - ctx_s); transpose into yctxT
                tmpc = atts.tile([NB, 512], F32, tag="tmpc", name=f"tmpc{t}")
                nc.vector.tensor_sub(tmpc[:], ctx4["e"][:], ctx4["s"][:])
                context4 = atts.tile([NB, 512], F32, tag="cx4", name=f"cx4_{t}")
                nc.vector.scalar_tensor_tensor(
                    context4[:], tmpc[:], plan0[:], ctx4["s"][:],
                    op0=ALU.mult, op1=ALU.add,
                )
                for hc in range(HC):
                    tps3 = tp.tile([128, 512], F32, tag="tp", name=f"tpc{t}{hc}")
                    nc.tensor.transpose(
                        tps3[:, 0:NB], context4[:, hc * 128:(hc + 1) * 128],
                        ident_f[0:NB, 0:NB],
                    )
                    nc.scalar.copy(yctxT[:, 4 + hc, t * NB:(t + 1) * NB], tps3[:, 0:NB])

            # x = WcT.T @ y_ctxT + bc
            xT_sb = mid.tile([128, HC, BSH], F32R)
            for kc in range(HC):
                ps = mm.tile([128, 512], F32, tag="mm")
                for j in range(EHC):
                    nc.tensor.matmul(
                        ps[:, 0:BSH],
                        wct_sb[:, j, kc * 128:(kc + 1) * 128],
                        yctxT[:, j, :],
                        start=(j == 0), stop=(j == EHC - 1),
                    )
                nc.scalar.activation(
                    xT_sb[:, kc, :], ps[:, 0:BSH], AF.Identity, bias=bc_sb[:, kc:kc + 1]
                )

            # AllGather x: [512, 32] -> [512, 256] (rank-major columns)
            nc.sync.dma_start(
                out=agx_in[:].rearrange("(k p b) -> p k b", p=128, b=BSH), in_=xT_sb[:]
            )
            nc.gpsimd.collective_compute(
                "AllGather", ALU.bypass,
                replica_groups=[list(range(NC))],
                ins=[agx_in[:]], outs=[agx_out[:]],
            )
            xfull_sb = mid.tile([128, HC, NC, BSH], F32R)
            agx_view = agx_out[:].rearrange("(r k p b) -> p k r b", r=NC, p=128, b=BSH)
            for k in range(HC):
                nc.sync.dma_start(out=xfull_sb[:, k, :, :], in_=agx_view[:, k, :, :])

            # ---- GRU gates (tensor-parallel over DSH rows) ----
            h_tiles = {}
            for si, (off, sz) in enumerate(SUBS):
                bsum_t = (bsum_hi, bsum_lo)[si]
                bih_t = (bih_hi, bih_lo)[si]
                bhh_t = (bhh_hi, bhh_lo)[si]
                phmyd_t = (phmyd_hi, phmyd_lo)[si]

                def gate_psum(g):
                    ps = mm.tile([128, 512], F32, tag="mm")
                    for k in range(HC):
                        nc.tensor.matmul(
                            ps[0:sz, 0:B], wih_sb[:, g, k, off:off + sz],
                            xfull_sb[:, k, :, :],
                            start=(k == 0), stop=False,
                        )
                    for k in range(DC):
                        nc.tensor.matmul(
                            ps[0:sz, 0:B], whh_sb[:, g, k, off:off + sz],
                            phT_sb[:, k, :],
                            start=False, stop=(k == DC - 1),
                        )
                    return ps

                ps_r = gate_psum(0)
                r_t = mid.tile([sz, B], F32, tag=f"r{si}")
                nc.scalar.activation(r_t[:], ps_r[0:sz, 0:B], AF.Sigmoid, bias=bsum_t[:sz, 0:1])
                ps_z = gate_psum(1)
                z_t = mid.tile([sz, B], F32, tag=f"z{si}")
                nc.scalar.activation(z_t[:], ps_z[0:sz, 0:B], AF.Sigmoid, bias=bsum_t[:sz, 1:2])

                ps_ghn = mm.tile([128, 512], F32, tag="mm")
                for k in range(DC):
                    nc.tensor.matmul(
                        ps_ghn[0:sz, 0:B], whh_sb[:, 2, k, off:off + sz], phT_sb[:, k, :],
                        start=(k == 0), stop=(k == DC - 1),
                    )
                ghn_t = mid.tile([sz, B], F32, tag=f"ghn{si}")
                nc.scalar.activation(ghn_t[:], ps_ghn[0:sz, 0:B], AF.Identity, bias=bhh_t[:sz, 2:3])

                ps_gxn = mm.tile([128, 512], F32, tag="mm")
                for k in range(HC):
                    nc.tensor.matmul(
                        ps_gxn[0:sz, 0:B], wih_sb[:, 2, k, off:off + sz], xfull_sb[:, k, :, :],
                        start=(k == 0), stop=(k == HC - 1),
                    )
                nc.vector.tensor_mul(ghn_t[:], r_t[:], ghn_t[:])
                npre_t = mid.tile([sz, B], F32, tag=f"npre{si}")
                nc.vector.tensor_add(npre_t[:], ghn_t[:], ps_gxn[0:sz, 0:B])
                n_t = mid.tile([sz, B], F32, tag=f"n{si}")
                nc.scalar.activation(n_t[:], npre_t[:], AF.Tanh, bias=bih_t[:sz, 2:3])

                # h = n + z * (h_prev - n)
                t2 = mid.tile([sz, B], F32, tag=f"t2{si}")
                nc.vector.tensor_sub(t2[:], phmyd_t[:sz, :], n_t[:])
                nc.vector.tensor_mul(t2[:], z_t[:], t2[:])
                h_t = mid.tile([sz, B], F32, tag=f"hh{si}")
                nc.vector.tensor_add(h_t[:], n_t[:], t2[:])
                h_tiles[si] = h_t
                nc.sync.dma_start(out=h_new_sh[off:off + sz, :], in_=h_t[:])
                h_r = mid.tile([sz, B], F32R, tag=f"hr{si}")
                nc.vector.tensor_copy(h_r[:], h_t[:])
                nc.sync.dma_start(
                    out=agh_in[off * B:(off + sz) * B].rearrange("(p b) -> p b", p=sz),
                    in_=h_r[:],
                )

            # AllGather h_new: [144,256] -> [1152,256] (rank-major rows = d-order)
            nc.gpsimd.collective_compute(
                "AllGather", ALU.bypass,
                replica_groups=[list(range(NC))],
                ins=[agh_in[:]], outs=[agh_out[:]],
            )
            hfull_sb = mid.tile([128, DC, B], F32R)
            nc.sync.dma_start(
                out=hfull_sb[:],
                in_=agh_out[:].rearrange("(k p b) -> p k b", p=128, b=B),
            )

            # ---- output projection: logitsT = WoutT.T @ h_full + bout ----
            for m in range(VC):
                msz = 128 if m < VC - 1 else 32
                wm = woutp.tile([128, DC, 128], F32R, tag="wout")
                nc.gpsimd.dma_start(
                    out=wm[:, :, 0:msz],
                    in_=woutt[:, m * 128:m * 128 + msz].rearrange("(k p) v -> p k v", p=128),
                )
                ps = mm.tile([128, 512], F32, tag="mm")
                for k in range(DC):
                    nc.tensor.matmul(
                        ps[0:msz, 0:B], wm[:, k, 0:msz], hfull_sb[:, k, :],
                        start=(k == 0), stop=(k == DC - 1),
                    )
                lo = loutp.tile([128, B], F32, tag="lout")
                nc.scalar.activation(
                    lo[0:msz, :], ps[0:msz, 0:B], AF.Identity, bias=bout_sb[:msz, m:m + 1]
                )
                nc.sync.dma_start(out=logitsT[m * 128:m * 128 + msz, :], in_=lo[0:msz, :])

    nc.compile()
    return nc


_CACHED = {}


def _get_nc():
    if "nc" not in _CACHED:
        _CACHED["nc"] = build()
    return _CACHED["nc"]


def kernel(prev_y, prev_h, equ_enc, sns_enc, z_sample, equ_mask, sns_mask,
           W1e, W2e, vte, W1s, W2s, vts, plan_W, plan_b, Wc, bc,
           w_ih, w_hh, b_ih, b_hh, Wout, bout):
    f = np.float32
    prev_y = np.asarray(prev_y, f)
    prev_h = np.asarray(prev_h, f)
    equ_enc = np.ascontiguousarray(np.asarray(equ_enc, f))
    sns_enc = np.ascontiguousarray(np.asarray(sns_enc, f))
    add_e = np.where(np.asarray(equ_mask), f(-1e30), f(0.0)).astype(f)
    add_s = np.where(np.asarray(sns_mask), f(-1e30), f(0.0)).astype(f)

    w1etT = np.ascontiguousarray(np.asarray(W1e, f).T)
    w1stT = np.ascontiguousarray(np.asarray(W1s, f).T)
    w2etT = np.ascontiguousarray(np.asarray(W2e, f).T)
    w2stT = np.ascontiguousarray(np.asarray(W2s, f).T)
    vte4 = np.ascontiguousarray(np.repeat(np.asarray(vte, f).reshape(H, 1), NB, axis=1))
    vts4 = np.ascontiguousarray(np.repeat(np.asarray(vts, f).reshape(H, 1), NB, axis=1))
    wctT = np.ascontiguousarray(np.asarray(Wc, f).T)
    planwT = np.ascontiguousarray(np.asarray(plan_W, f).T)
    planb2 = np.asarray(plan_b, f).reshape(2, 1)
    bc1 = np.asarray(bc, f)
    phT = np.ascontiguousarray(prev_h.T)
    pyT = np.ascontiguousarray(prev_y.T)
    wih3 = np.asarray(w_ih, f).reshape(3, D, H)
    whh3 = np.asarray(w_hh, f).reshape(3, D, D)
    bih3 = np.asarray(b_ih, f).reshape(3, D)
    bhh3 = np.asarray(b_hh, f).reshape(3, D)
    Wout = np.asarray(Wout, f)
    bout = np.asarray(bout, f)

    diag4_np = np.zeros((NB, NB * N), np.float32)
    for j in range(NB):
        diag4_np[j, j * N:(j + 1) * N] = 1.0

    in_maps = []
    for c in range(NC):
        b0 = c * BSH
        d0 = c * DSH
        v0 = c * VSH
        in_maps.append({
            "enc_e": equ_enc[b0:b0 + BSH],
            "enc_s": sns_enc[b0:b0 + BSH],
            "mask_e": add_e[b0:b0 + BSH],
            "mask_s": add_s[b0:b0 + BSH],
            "w1et": w1etT, "w1st": w1stT,
            "w2et": w2etT, "w2st": w2stT,
            "vte4": vte4, "vts4": vts4,
            "wct": wctT,
            "planwt": planwT, "planb": planb2,
            "bc": bc1,
            "ph_T": phT,
            "ph_myb": np.ascontiguousarray(phT[:, b0:b0 + BSH]),
            "ph_myd": np.ascontiguousarray(phT[d0:d0 + DSH, :]),
            "pyt": np.ascontiguousarray(pyT[:, b0:b0 + BSH]),
            "wih": np.ascontiguousarray(wih3[:, d0:d0 + DSH, :].transpose(0, 2, 1)),
            "whh": np.ascontiguousarray(whh3[:, d0:d0 + DSH, :].transpose(0, 2, 1)),
            "bih": np.ascontiguousarray(bih3[:, d0:d0 + DSH]),
            "bhh": np.ascontiguousarray(bhh3[:, d0:d0 + DSH]),
            "woutt": np.ascontiguousarray(Wout[v0:v0 + VSH].T),
            "diag4": diag4_np,
            "bout": np.ascontiguousarray(bout[v0:v0 + VSH]),
        })

    nc = _get_nc()
    res = run_bass_kernel_spmd(nc, in_maps, core_ids=list(range(NC)))

    logits = np.concatenate(
        [res.results[c]["logitsT"].T for c in range(NC)], axis=1
    ).astype(f)
    h_newT = np.concatenate([res.results[c]["h_new_sh"] for c in range(NC)], axis=0)
    plan = np.concatenate([res.results[c]["plan_sh"] for c in range(NC)], axis=0).astype(f)

    dec_output = logits[:, None, :]
    dec_hidden = np.ascontiguousarray(h_newT.T)[None]
    return dec_output, dec_hidden, plan


# revision 14
# speedup vs baseline: 1.4123x; 1.4123x over previous
"""DecoderVAE single-step fused kernel for one TRN2 chip (8 NeuronCores).

Sharding: data-parallel over batch (32/core) for attention + decoder front;
tensor-parallel GRU (144 state rows/core) and output projection (4000 vocab
rows/core), stitched by two on-device AllGathers (x: 512x256, h: 1152x256).

Attention runs in bf16 on a host-pre-transposed encoder layout (no on-chip
transposes); everything else in float32r (full-rate PE, ~2e-4 rel err).
All bulk DRAM->SBUF moves use host-prearranged partition-major layouts so
each DMA is one descriptor per partition.
"""
import sys
import numpy as np

for _p in ("/opt/trn_rl_repo", "/root/.axon_site/_ro/trn_rl_repo"):
    if _p not in sys.path:
        sys.path.append(_p)

import concourse.bacc as bacc
import concourse.mybir as mybir
from concourse import tile
from concourse.bass_utils import run_bass_kernel_spmd
from concourse.masks import make_identity
import concourse.bass as bass


def _bcast_mid(ap_obj, axis, count):
    """Insert a 0-stride broadcast dim at `axis` of an AP."""
    dims = [list(d) for d in ap_obj.ap]
    dims.insert(axis, [0, count])
    return bass.AP(tensor=ap_obj.tensor, offset=ap_obj.offset, ap=dims)

dt = mybir.dt
AF = mybir.ActivationFunctionType
ALU = mybir.AluOpType
AX = mybir.AxisListType

B = 256
H = 512
E = 512
D = 1152
V = 32000
N = 128

NC = 8
BSH = B // NC        # 32
NB = 4               # batch rows per attention tile
NT = BSH // NB       # 8
HC = H // 128        # 4
EHC = (E + H) // 128  # 8
DC = D // 128        # 9
DSH = D // NC        # 144
VSH = V // NC        # 4000
VT = 8               # vocab tiles per core
VS = VSH // VT       # 500
SUBS = ((0, 128), (128, 16))

F32 = dt.float32
F32R = dt.float32r
BF16 = dt.bfloat16


def build():
    nc = bacc.Bacc(None, target_bir_lowering=False, num_devices=NC)

    def param(name, shape):
        return nc.declare_dram_parameter(name, list(shape), F32, isOutput=False)

    # host-prearranged layouts (partition-major)
    encT_e = param("encT_e", (NT, 128, HC * NB * N))   # [t][h'][kc,b,n]
    encT_s = param("encT_s", (NT, 128, HC * NB * N))
    mask_e = param("mask_e", (NB, NT, N))
    mask_s = param("mask_s", (NB, NT, N))
    w1et = param("w1et", (128, HC, H))                 # [p][kc][hout]
    w1st = param("w1st", (128, HC, H))
    w2et = param("w2et", (128, DC, H))
    w2st = param("w2st", (128, DC, H))
    vte4 = param("vte4", (128, HC, NB))
    vts4 = param("vts4", (128, HC, NB))
    wct = param("wct", (128, EHC, H))
    planwt = param("planwt", (128, DC, 2))
    planb = param("planb", (2, 1))
    bc = param("bc", (128, HC))
    ph_T = param("ph_T", (128, DC, B))
    ph_myb = param("ph_myb", (128, DC, BSH))
    ph_myd = param("ph_myd", (DSH, B))
    pyt = param("pyt", (128, 4, BSH))
    wih = param("wih", (128, 3, HC, DSH))
    whh = param("whh", (128, 3, DC, DSH))
    bih = param("bih", (DSH, 3))
    bhh = param("bhh", (DSH, 3))
    woutt = param("woutt", (VT, 128, DC * VS))         # [vt][p][k,vs]
    bout = param("bout", (VT, VS))
    diag4 = param("diag4", (NB, NB * N))

    logits_o = nc.declare_dram_parameter("logits", [B, VSH], F32, isOutput=True)
    h_new_sh = nc.declare_dram_parameter("h_new_sh", [DSH, B], F32, isOutput=True)
    plan_sh = nc.declare_dram_parameter("plan_sh", [BSH, 2], F32, isOutput=True)

    with tile.TileContext(nc) as tc:
        with (
            tc.tile_pool(name="wres", bufs=1) as wres,
            tc.tile_pool(name="w2p", bufs=1) as w2p,
            tc.tile_pool(name="encp", bufs=3) as encp,
            tc.tile_pool(name="tanhp", bufs=2) as tanhp,
            tc.tile_pool(name="atts", bufs=2) as atts,
            tc.tile_pool(name="mid", bufs=1) as mid,
            tc.tile_pool(name="woutp", bufs=2) as woutp,
            tc.tile_pool(name="loutp", bufs=2) as loutp,
            tc.tile_pool(name="mm", bufs=4, space="PSUM") as mm,
            tc.tile_pool(name="up", bufs=2, space="PSUM") as up,
            tc.tile_pool(name="tp", bufs=1, space="PSUM") as tp,
            tc.tile_pool(name="dram", bufs=1, space="DRAM") as dram,
        ):
            # ---- collective bounce buffers ----
            agx_in = dram.tile([H * BSH], F32R)
            agx_out = dram.tile([H * B], F32R, addr_space="Shared")
            agh_in = dram.tile([DSH * B], F32R)
            agh_out = dram.tile([D * B], F32R, addr_space="Shared")

            # ---- resident weights / constants ----
            ident_f = wres.tile([128, 128], F32)
            make_identity(nc, ident_f[:])
            ones4 = wres.tile([NB, 128], BF16)
            nc.vector.memset(ones4[:], 1.0)
            ones1r = wres.tile([1, 128], F32R)
            ones1f = wres.tile([1, 128], F32)
            nc.vector.memset(ones1f[:], 1.0)
            nc.vector.tensor_copy(ones1r[:], ones1f[:])

            w1et_sb = wres.tile([128, HC, H], BF16)
            nc.gpsimd.dma_start(out=w1et_sb[:], in_=w1et[:])
            w1st_sb = wres.tile([128, HC, H], BF16)
            nc.gpsimd.dma_start(out=w1st_sb[:], in_=w1st[:])
            vte4_sb = wres.tile([128, HC, NB], BF16)
            nc.gpsimd.dma_start(out=vte4_sb[:], in_=vte4[:])
            vts4_sb = wres.tile([128, HC, NB], BF16)
            nc.gpsimd.dma_start(out=vts4_sb[:], in_=vts4[:])
            wct_sb = wres.tile([128, EHC, H], F32R)
            nc.gpsimd.dma_start(out=wct_sb[:], in_=wct[:])
            planwt_sb = wres.tile([128, DC, 2], F32R)
            nc.gpsimd.dma_start(out=planwt_sb[:], in_=planwt[:])
            phT_sb = wres.tile([128, DC, B], F32R)
            nc.gpsimd.dma_start(out=phT_sb[:], in_=ph_T[:])
            phmyb_sb = wres.tile([128, DC, BSH], F32R)
            nc.gpsimd.dma_start(out=phmyb_sb[:], in_=ph_myb[:])
            wih_sb = wres.tile([128, 3, HC, DSH], F32R)
            nc.gpsimd.dma_start(out=wih_sb[:], in_=wih[:])
            whh_sb = wres.tile([128, 3, DC, DSH], F32R)
            nc.gpsimd.dma_start(out=whh_sb[:], in_=whh[:])

            maskE_sb = wres.tile([NB, NT, N], F32)
            nc.sync.dma_start(out=maskE_sb[:], in_=mask_e[:])
            maskS_sb = wres.tile([NB, NT, N], F32)
            nc.sync.dma_start(out=maskS_sb[:], in_=mask_s[:])
            bc_sb = wres.tile([128, HC], F32)
            nc.sync.dma_start(out=bc_sb[:], in_=bc[:])
            planb_sb = wres.tile([2, 1], F32)
            nc.sync.dma_start(out=planb_sb[:], in_=planb[:])
            phmyd_hi = wres.tile([128, B], F32)
            nc.sync.dma_start(out=phmyd_hi[:], in_=ph_myd[0:128, :])
            phmyd_lo = wres.tile([16, B], F32)
            nc.sync.dma_start(out=phmyd_lo[:], in_=ph_myd[128:144, :])
            bih_hi = wres.tile([128, 3], F32)
            nc.sync.dma_start(out=bih_hi[:], in_=bih[0:128, :])
            bih_lo = wres.tile([16, 3], F32)
            nc.sync.dma_start(out=bih_lo[:], in_=bih[128:144, :])
            bhh_hi = wres.tile([128, 3], F32)
            nc.sync.dma_start(out=bhh_hi[:], in_=bhh[0:128, :])
            bhh_lo = wres.tile([16, 3], F32)
            nc.sync.dma_start(out=bhh_lo[:], in_=bhh[128:144, :])
            bsum_hi = wres.tile([128, 3], F32)
            nc.vector.tensor_add(bsum_hi[:], bih_hi[:], bhh_hi[:])
            bsum_lo = wres.tile([16, 3], F32)
            nc.vector.tensor_add(bsum_lo[:], bih_lo[:], bhh_lo[:])
            diag4_sb = wres.tile([NB, NB * N], F32)
            nc.sync.dma_start(out=diag4_sb[:], in_=diag4[:])

            # ---- dt (attention query proj): dtT[hout, b] per encoder ----
            w2et_sb = w2p.tile([128, DC, H], F32R, tag="w2")
            nc.gpsimd.dma_start(out=w2et_sb[:], in_=w2et[:])
            dtTe = wres.tile([128, HC, BSH], F32)
            for hc in range(HC):
                ps = mm.tile([128, 512], F32, tag="mm", name=f"dte{hc}")
                for k in range(DC):
                    nc.tensor.matmul(
                        ps[:, 0:BSH], w2et_sb[:, k, hc * 128:(hc + 1) * 128],
                        phmyb_sb[:, k, :], start=(k == 0), stop=(k == DC - 1),
                    )
                nc.scalar.copy(dtTe[:, hc, :], ps[:, 0:BSH])
            w2st_sb = w2p.tile([128, DC, H], F32R, tag="w2")
            nc.gpsimd.dma_start(out=w2st_sb[:], in_=w2st[:])
            dtTs = wres.tile([128, HC, BSH], F32)
            for hc in range(HC):
                ps = mm.tile([128, 512], F32, tag="mm", name=f"dts{hc}")
                for k in range(DC):
                    nc.tensor.matmul(
                        ps[:, 0:BSH], w2st_sb[:, k, hc * 128:(hc + 1) * 128],
                        phmyb_sb[:, k, :], start=(k == 0), stop=(k == DC - 1),
                    )
                nc.scalar.copy(dtTs[:, hc, :], ps[:, 0:BSH])

            # ---- plan: softmax over 2 -> sigmoid(diff), in [4, NT] layout ----
            ps = mm.tile([128, 512], F32, tag="mm", name="plps")
            for k in range(DC):
                nc.tensor.matmul(
                    ps[0:2, 0:BSH], planwt_sb[:, k, :], phmyb_sb[:, k, :],
                    start=(k == 0), stop=(k == DC - 1),
                )
            pl_sb = mid.tile([2, BSH], F32)
            nc.scalar.activation(pl_sb[:], ps[0:2, 0:BSH], AF.Identity, bias=planb_sb[:])
            plan0_mat = wres.tile([NB, NT], F32)
            plan1_mat = wres.tile([NB, NT], F32)
            for t in range(NT):
                tps = tp.tile([128, 512], F32, tag="tp", name=f"plt{t}")
                nc.tensor.transpose(
                    tps[0:NB, 0:2], pl_sb[:, t * NB:(t + 1) * NB], ident_f[0:2, 0:2]
                )
                planT4 = atts.tile([NB, 2], F32, tag="pT", name=f"pT{t}")
                nc.scalar.copy(planT4[:], tps[0:NB, 0:2])
                d01 = atts.tile([NB, 1], F32, tag="d01", name=f"d01_{t}")
                nc.vector.tensor_sub(d01[:], planT4[:, 0:1], planT4[:, 1:2])
                nc.scalar.activation(plan0_mat[:, t:t + 1], d01[:], AF.Sigmoid)
                nc.scalar.activation(plan1_mat[:, t:t + 1], plan0_mat[:, t:t + 1],
                                     AF.Identity, bias=1.0, scale=-1.0)
                plan_out4 = atts.tile([NB, 2], F32, tag="po", name=f"po_{t}")
                nc.vector.tensor_copy(plan_out4[:, 0:1], plan0_mat[:, t:t + 1])
                nc.vector.tensor_copy(plan_out4[:, 1:2], plan1_mat[:, t:t + 1])
                nc.sync.dma_start(out=plan_sh[t * NB:(t + 1) * NB, :], in_=plan_out4[:])

            # y_ctxT = [prev_yT ; contextT]; context columns filled per tile
            yctxT = mid.tile([128, EHC, BSH], F32R)
            nc.gpsimd.dma_start(out=yctxT[:, 0:4, :], in_=pyt[:])

            # ---- attention ----
            srcs = {
                "e": (encT_e, w1et_sb, vte4_sb, dtTe, maskE_sb, plan0_mat),
                "s": (encT_s, w1st_sb, vts4_sb, dtTs, maskS_sb, plan1_mat),
            }
            for t in range(NT):
                cte = {}
                for key in ("e", "s"):
                    encT_p, w1_sb, vt_sb, dtT, mask_sb, plan_mat = srcs[key]
                    encT_sb = encp.tile([128, HC, NB, N], BF16, tag="encT",
                                        name=f"encT_{key}{t}")
                    nc.gpsimd.dma_start(
                        out=encT_sb[:],
                        in_=encT_p[t].rearrange("p (k b n) -> p k b n", k=HC, b=NB),
                    )
                    T_sb = tanhp.tile([128, HC, NB, N], BF16, tag="tanh",
                                      name=f"T_{key}{t}")
                    for hc in range(HC):
                        ps_et = mm.tile([128, 512], F32, tag="mm", name=f"et_{key}{t}{hc}")
                        for kc in range(HC):
                            nc.tensor.matmul(
                                ps_et[:], w1_sb[:, kc, hc * 128:(hc + 1) * 128],
                                encT_sb[:, kc, :, :],
                                start=(kc == 0), stop=(kc == HC - 1),
                            )
                        # T_pre = et + dt (dt broadcast along n via 0-stride AP)
                        tpre = tanhp.tile([128, NB, N], BF16, tag="tpre",
                                          name=f"tpre_{key}{t}{hc}")
                        dtT_bc = _bcast_mid(dtT[:, hc, t * NB:(t + 1) * NB], 2, N)
                        nc.vector.tensor_add(
                            tpre[:], ps_et[:].rearrange("p (b n) -> p b n", b=NB), dtT_bc
                        )
                        nc.scalar.activation(T_sb[:, hc, :, :], tpre[:], AF.Tanh)
                    # u = vt . T (4 identical rows; block-select)
                    ups = up.tile([NB, 512], F32, tag="u", name=f"u_{key}{t}")
                    for hc in range(HC):
                        nc.tensor.matmul(
                            ups[:], vt_sb[:, hc, :], T_sb[:, hc, :, :],
                            start=(hc == 0), stop=(hc == HC - 1),
                        )
                    usel = atts.tile([NB, NB * N], F32, tag="usel", name=f"usel_{key}{t}")
                    nc.vector.tensor_mul(usel[:], ups[:], diag4_sb[:])
                    u4 = atts.tile([NB, N], F32, tag="u4", name=f"u4_{key}{t}")
                    nc.vector.reduce_sum(
                        u4[:], usel[:].rearrange("p (b n) -> p n b", b=NB), axis=AX.X
                    )
                    nc.vector.tensor_add(u4[:], u4[:], mask_sb[:, t, :])
                    # softmax (no max-subtraction: |u| is small) + plan weighting
                    aw = atts.tile([NB, N], F32, tag="aw", name=f"aw_{key}{t}")
                    sums = atts.tile([NB, 1], F32, tag="sm", name=f"sm_{key}{t}")
                    nc.scalar.activation(aw[:], u4[:], AF.Exp, accum_out=sums[:])
                    rinv = atts.tile([NB, 1], F32, tag="ri", name=f"ri_{key}{t}")
                    nc.vector.reciprocal(rinv[:], sums[:])
                    nc.vector.tensor_scalar(
                        aw[:], aw[:], rinv[:], plan_mat[:, t:t + 1],
                        op0=ALU.mult, op1=ALU.mult,
                    )
                    # block-diagonal aw, broadcast to 128 partitions via PE
                    awblk = atts.tile([NB, NB * N], BF16, tag="awblk",
                                      name=f"awblk_{key}{t}")
                    nc.vector.tensor_mul(
                        awblk[:].rearrange("p (b n) -> p b n", b=NB),
                        _bcast_mid(aw[:], 1, NB),
                        diag4_sb[:].rearrange("p (b n) -> p b n", b=NB),
                    )
                    ps_bc = mm.tile([128, 512], F32, tag="mm", name=f"awbc_{key}{t}")
                    nc.tensor.matmul(ps_bc[:], ones4[:], awblk[:], start=True, stop=True)
                    # ctx columns: reduce_n( aw_bc * encT ) per kc
                    ctile = atts.tile([128, HC, NB], F32, tag=f"ct{key}",
                                      name=f"ct_{key}{t}")
                    for kc in range(HC):
                        prod = atts.tile([128, NB, N], BF16, tag="prod",
                                         name=f"prod_{key}{t}{kc}")
                        nc.vector.scalar_tensor_tensor(
                            prod[:].rearrange("p b n -> p (b n)"), ps_bc[:], 1.0,
                            encT_sb[:, kc, :, :].rearrange("p b n -> p (b n)"),
                            op0=ALU.mult, op1=ALU.mult,
                        )
                        nc.vector.reduce_sum(ctile[:, kc, :], prod[:], axis=AX.X)
                    cte[key] = ctile
                # context columns -> yctxT
                for kc in range(HC):
                    nc.vector.tensor_add(
                        yctxT[:, 4 + kc, t * NB:(t + 1) * NB],
                        cte["e"][:, kc, :], cte["s"][:, kc, :],
                    )

            # ---- x = WcT.T @ y_ctxT + bc ----
            xT_sb = mid.tile([128, HC, BSH], F32R)
            for kc in range(HC):
                ps = mm.tile([128, 512], F32, tag="mm", name=f"x{kc}")
                for j in range(EHC):
                    nc.tensor.matmul(
                        ps[:, 0:BSH], wct_sb[:, j, kc * 128:(kc + 1) * 128],
                        yctxT[:, j, :], start=(j == 0), stop=(j == EHC - 1),
                    )
                nc.scalar.activation(
                    xT_sb[:, kc, :], ps[:, 0:BSH], AF.Identity, bias=bc_sb[:, kc:kc + 1]
                )

            # ---- AllGather x ----
            nc.sync.dma_start(
                out=agx_in[:].rearrange("(k p b) -> p k b", p=128, b=BSH), in_=xT_sb[:]
            )
            nc.gpsimd.collective_compute(
                "AllGather", ALU.bypass, replica_groups=[list(range(NC))],
                ins=[agx_in[:]], outs=[agx_out[:]],
            )
            xfull_sb = mid.tile([128, HC, NC, BSH], F32R)
            agx_view = agx_out[:].rearrange("(r k p b) -> p k r b", r=NC, p=128, b=BSH)
            for k in range(HC):
                nc.sync.dma_start(out=xfull_sb[:, k, :, :], in_=agx_view[:, k, :, :])

            # ---- GRU gates (tensor-parallel over DSH rows) ----
            for si, (off, sz) in enumerate(SUBS):
                bsum_t = (bsum_hi, bsum_lo)[si]
                bih_t = (bih_hi, bih_lo)[si]
                bhh_t = (bhh_hi, bhh_lo)[si]
                phmyd_t = (phmyd_hi, phmyd_lo)[si]

                def gate_psum(g, name):
                    ps = mm.tile([128, 512], F32, tag="mm", name=name)
                    for k in range(HC):
                        nc.tensor.matmul(
                            ps[0:sz, 0:B], wih_sb[:, g, k, off:off + sz],
                            xfull_sb[:, k, :, :], start=(k == 0), stop=False,
                        )
                    for k in range(DC):
                        nc.tensor.matmul(
                            ps[0:sz, 0:B], whh_sb[:, g, k, off:off + sz],
                            phT_sb[:, k, :], start=False, stop=(k == DC - 1),
                        )
                    return ps

                ps_r = gate_psum(0, f"gr{si}")
                r_t = mid.tile([sz, B], F32, tag=f"r{si}")
                nc.scalar.activation(r_t[:], ps_r[0:sz, 0:B], AF.Sigmoid,
                                     bias=bsum_t[:sz, 0:1])
                ps_z = gate_psum(1, f"gz{si}")
                z_t = mid.tile([sz, B], F32, tag=f"z{si}")
                nc.scalar.activation(z_t[:], ps_z[0:sz, 0:B], AF.Sigmoid,
                                     bias=bsum_t[:sz, 1:2])

                ps_ghn = mm.tile([128, 512], F32, tag="mm", name=f"ghn{si}")
                for k in range(DC):
                    nc.tensor.matmul(
                        ps_ghn[0:sz, 0:B], whh_sb[:, 2, k, off:off + sz],
                        phT_sb[:, k, :], start=(k == 0), stop=(k == DC - 1),
                    )
                ghn_t = mid.tile([sz, B], F32, tag=f"ghn{si}")
                nc.scalar.activation(ghn_t[:], ps_ghn[0:sz, 0:B], AF.Identity,
                                     bias=bhh_t[:sz, 2:3])
                ps_gxn = mm.tile([128, 512], F32, tag="mm", name=f"gxn{si}")
                for k in range(HC):
                    nc.tensor.matmul(
                        ps_gxn[0:sz, 0:B], wih_sb[:, 2, k, off:off + sz],
                        xfull_sb[:, k, :, :], start=(k == 0), stop=(k == HC - 1),
                    )
                nc.vector.tensor_mul(ghn_t[:], r_t[:], ghn_t[:])
                npre_t = mid.tile([sz, B], F32, tag=f"npre{si}")
                nc.vector.tensor_add(npre_t[:], ghn_t[:], ps_gxn[0:sz, 0:B])
                n_t = mid.tile([sz, B], F32, tag=f"n{si}")
                nc.scalar.activation(n_t[:], npre_t[:], AF.Tanh, bias=bih_t[:sz, 2:3])

                t2 = mid.tile([sz, B], F32, tag=f"t2{si}")
                nc.vector.tensor_sub(t2[:], phmyd_t[:sz, :], n_t[:])
                nc.vector.tensor_mul(t2[:], z_t[:], t2[:])
                h_t = mid.tile([sz, B], F32, tag=f"hh{si}")
                nc.vector.tensor_add(h_t[:], n_t[:], t2[:])
                nc.sync.dma_start(out=h_new_sh[off:off + sz, :], in_=h_t[:])
                h_r = mid.tile([sz, B], F32R, tag=f"hr{si}")
                nc.vector.tensor_copy(h_r[:], h_t[:])
                nc.sync.dma_start(
                    out=agh_in[off * B:(off + sz) * B].rearrange("(p b) -> p b", p=sz),
                    in_=h_r[:],
                )

            # ---- AllGather h ----
            nc.gpsimd.collective_compute(
                "AllGather", ALU.bypass, replica_groups=[list(range(NC))],
                ins=[agh_in[:]], outs=[agh_out[:]],
            )
            hfull_sb = mid.tile([128, DC, B], F32R)
            nc.sync.dma_start(
                out=hfull_sb[:], in_=agh_out[:].rearrange("(k p b) -> p k b", p=128, b=B),
            )

            # ---- logits = h_full @ WoutT + bout  (h stationary, natural layout) ----
            for vt in range(VT):
                wm = woutp.tile([128, DC, VS], F32R, tag="wout", name=f"wm{vt}")
                nc.gpsimd.dma_start(
                    out=wm[:], in_=woutt[vt].rearrange("p (k v) -> p k v", k=DC)
                )
                bo = loutp.tile([1, VS], F32R, tag="bo", name=f"bo{vt}")
                nc.gpsimd.dma_start(out=bo[:], in_=bout[vt:vt + 1, :])
                for bchunk in range(2):
                    ps = mm.tile([128, 512], F32, tag="mm", name=f"lg{vt}{bchunk}")
                    for k in range(DC):
                        nc.tensor.matmul(
                            ps[:, 0:VS],
                            hfull_sb[:, k, bchunk * 128:(bchunk + 1) * 128],
                            wm[:, k, :], start=(k == 0), stop=False,
                        )
                    nc.tensor.matmul(
                        ps[:, 0:VS], ones1r[:], bo[:],
                        start=False, stop=True,
                    )
                    lo = loutp.tile([128, VS], F32, tag="lout", name=f"lo{vt}{bchunk}")
                    nc.scalar.copy(lo[:], ps[:, 0:VS])
                    nc.sync.dma_start(
                        out=logits_o[bchunk * 128:(bchunk + 1) * 128,
                                     vt * VS:(vt + 1) * VS],
                        in_=lo[:],
                    )

    nc.compile()
    return nc


_CACHED = {}


def _get_nc():
    if "nc" not in _CACHED:
        _CACHED["nc"] = build()
    return _CACHED["nc"]


def _pm(a, p=128):
    """(k*p, rest...) -> (p, k, rest...) partition-major, contiguous"""
    a = np.ascontiguousarray(a)
    k = a.shape[0] // p
    a = a.reshape(k, p, *a.shape[1:])
    return np.ascontiguousarray(np.moveaxis(a, 1, 0))


def kernel(prev_y, prev_h, equ_enc, sns_enc, z_sample, equ_mask, sns_mask,
           W1e, W2e, vte, W1s, W2s, vts, plan_W, plan_b, Wc, bc,
           w_ih, w_hh, b_ih, b_hh, Wout, bout):
    f = np.float32
    prev_y = np.asarray(prev_y, f)
    prev_h = np.asarray(prev_h, f)
    equ_enc = np.asarray(equ_enc, f)
    sns_enc = np.asarray(sns_enc, f)
    add_e = np.where(np.asarray(equ_mask), f(-1e30), f(0.0)).astype(f)
    add_s = np.where(np.asarray(sns_mask), f(-1e30), f(0.0)).astype(f)

    w1etT = _pm(np.asarray(W1e, f).T)          # (128, 4, 512)
    w1stT = _pm(np.asarray(W1s, f).T)
    w2etT = _pm(np.asarray(W2e, f).T)          # (128, 9, 512)
    w2stT = _pm(np.asarray(W2s, f).T)
    vte4 = _pm(np.repeat(np.asarray(vte, f).reshape(H, 1), NB, axis=1))
    vts4 = _pm(np.repeat(np.asarray(vts, f).reshape(H, 1), NB, axis=1))
    wctT = _pm(np.asarray(Wc, f).T)            # (128, 8, 512)
    planwT = _pm(np.asarray(plan_W, f).T)      # (128, 9, 2)
    planb2 = np.asarray(plan_b, f).reshape(2, 1)
    bc1 = np.ascontiguousarray(np.asarray(bc, f).reshape(HC, 128).T)  # (128, 4)
    phT = np.ascontiguousarray(prev_h.T)       # (1152, 256)
    phT_pm = _pm(phT)                          # (128, 9, 256)
    pyT = np.ascontiguousarray(prev_y.T)       # (512, 256)
    wih3 = np.asarray(w_ih, f).reshape(3, D, H)
    whh3 = np.asarray(w_hh, f).reshape(3, D, D)
    bih3 = np.asarray(b_ih, f).reshape(3, D)
    bhh3 = np.asarray(b_hh, f).reshape(3, D)
    Wout = np.asarray(Wout, f)
    bout = np.asarray(bout, f)

    diag4_np = np.zeros((NB, NB * N), f)
    for j in range(NB):
        diag4_np[j, j * N:(j + 1) * N] = 1.0

    in_maps = []
    for c in range(NC):
        b0 = c * BSH
        d0 = c * DSH
        v0 = c * VSH

        def enc_tiles(enc):
            e = enc[b0:b0 + BSH]                       # (32, 128, 512)
            e = e.reshape(NT, NB, N, HC, 128)          # t, b, n, kc, p
            e = e.transpose(0, 4, 3, 1, 2)             # t, p, kc, b, n
            return np.ascontiguousarray(e.reshape(NT, 128, HC * NB * N))

        wt = Wout[v0:v0 + VSH].T                       # (1152, 4000)
        wt = wt.reshape(DC, 128, VT, VS)               # k, p, vt, vs
        wt = wt.transpose(2, 1, 0, 3)                  # vt, p, k, vs
        wt = np.ascontiguousarray(wt.reshape(VT, 128, DC * VS))

        wih_c = wih3[:, d0:d0 + DSH, :].transpose(0, 2, 1)   # (3, 512, 144)
        wih_pm = np.ascontiguousarray(
            np.moveaxis(wih_c.reshape(3, HC, 128, DSH), 2, 0))  # (128, 3, 4, 144)
        whh_c = whh3[:, d0:d0 + DSH, :].transpose(0, 2, 1)   # (3, 1152, 144)
        whh_pm = np.ascontiguousarray(
            np.moveaxis(whh_c.reshape(3, DC, 128, DSH), 2, 0))  # (128, 3, 9, 144)

        in_maps.append({
            "encT_e": enc_tiles(equ_enc),
            "encT_s": enc_tiles(sns_enc),
            "mask_e": np.ascontiguousarray(
                add_e[b0:b0 + BSH].reshape(NT, NB, N).transpose(1, 0, 2)),
            "mask_s": np.ascontiguousarray(
                add_s[b0:b0 + BSH].reshape(NT, NB, N).transpose(1, 0, 2)),
            "w1et": w1etT, "w1st": w1stT,
            "w2et": w2etT, "w2st": w2stT,
            "vte4": vte4, "vts4": vts4,
            "wct": wctT,
            "planwt": planwT, "planb": planb2,
            "bc": bc1,
            "ph_T": phT_pm,
            "ph_myb": _pm(phT[:, b0:b0 + BSH]),
            "ph_myd": np.ascontiguousarray(phT[d0:d0 + DSH, :]),
            "pyt": _pm(pyT[:, b0:b0 + BSH]),
            "wih": wih_pm, "whh": whh_pm,
            "bih": np.ascontiguousarray(bih3[:, d0:d0 + DSH].T),
            "bhh": np.ascontiguousarray(bhh3[:, d0:d0 + DSH].T),
            "woutt": wt,
            "bout": np.ascontiguousarray(bout[v0:v0 + VSH].reshape(VT, VS)),
            "diag4": diag4_np,
        })

    nc = _get_nc()
    res = run_bass_kernel_spmd(nc, in_maps, core_ids=list(range(NC)))

    logits = np.concatenate(
        [res.results[c]["logits"] for c in range(NC)], axis=1
    ).astype(f)
    h_newT = np.concatenate([res.results[c]["h_new_sh"] for c in range(NC)], axis=0)
    plan = np.concatenate([res.results[c]["plan_sh"] for c in range(NC)], axis=0).astype(f)

    dec_output = logits[:, None, :]
    dec_hidden = np.ascontiguousarray(h_newT.T)[None]
    return dec_output, dec_hidden, plan
